# revision 45
# baseline (speedup 1.0000x reference)
"""Multi-head causal self-attention (B=4, T=2048, C=768, H=12) on 8 trn2 cores.

Sharding: core c handles batch b = c//2 and head-group hg = c%2 (6 heads each).
Host sums the output-projection partials per batch, transposes back, and adds
b_o. No cross-core collectives.

Final: single software-pipelined attention loop across all 6 heads
(PV matmuls run two stages behind their scores/exp, so the PE never waits on
ACT exp or Pool masking); QKV chunk groups and output-projection units are
sprinkled into ACT-paced attention stages as PE filler work; the output
projection is split into a heads-0-3 partial (computed during head 4 as
filler, shipped separately and summed on the host) and a heads-4-5 remainder
(tiny endgame tail); inputs arrive pre-transposed/pre-cast from the host
so there are zero PE transposes; the QK projection runs as fp8e4m3
DoubleRow matmuls (x and Wqk shipped in fp8, 256-channel contraction per
chunk at 0.5 cycles/column -- 4x fewer PE cycles than bf16, ~55k cycles
saved) while the V projection stays bf16 so v (which y is linear in) is
unquantized; V is built in natural [token, feature]
layout with per-head [64 v | 1 ones] groups giving PV its softmax denominator
row for free; normalization reads straight from PSUM (DVE reciprocal + Pool
partition-broadcast + DVE multiply).
"""

import math
import os
from collections import deque

import numpy as np
import ml_dtypes

import concourse.bass as bass
from concourse import bacc
import concourse.mybir as mybir
import concourse.tile as tile
from concourse import bass_utils
from concourse.bass import ts
from concourse.masks import make_identity

F32 = mybir.dt.float32
BF16 = mybir.dt.bfloat16
F8 = mybir.dt.float8e4

P = 128
T = 2048          # sequence length
C = 768           # embed dim
CS = C // P       # 6 contraction chunks
HL = 6            # heads per core
HD = 64           # head dim
J = HL * HD       # 384 local y-feature dim
JS = J // P       # 3
OQK = 2 * J // P  # 6 o-blocks of the local W_qk slice (q rows then k rows)
OUTB = C // P     # 6 output row blocks
TT = T // 512     # 4 column tiles of 512
TB = T // P       # 16 token blocks
VG = HD + 1       # 65: per-head v columns + ones column


def _build_bass():
    nc = bacc.Bacc("TRN2", target_bir_lowering=False, debug=False)
    xt_d = nc.dram_tensor("xt", [C, T], BF16, kind="ExternalInput").ap()
    xt8_d = nc.dram_tensor("xt8", [C, T], F8, kind="ExternalInput").ap()
    wqk8_d = nc.dram_tensor("wqk8", [C, 2 * J], F8, kind="ExternalInput").ap()
    wv_d = nc.dram_tensor("wv", [C, J], BF16, kind="ExternalInput").ap()
    wo_d = nc.dram_tensor("wo", [J, C], BF16, kind="ExternalInput").ap()
    bqk_d = nc.dram_tensor("bqk", [2 * J], F32, kind="ExternalInput").ap()
    bv_d = nc.dram_tensor("bv", [J], F32, kind="ExternalInput").ap()
    outa_d = nc.dram_tensor("outa", [C, T], BF16, kind="ExternalOutput").ap()
    outb_d = nc.dram_tensor("outb", [C, T], BF16, kind="ExternalOutput").ap()

    with tile.TileContext(nc) as tc, nc.allow_low_precision(
        reason="fp8 QK projection + bf16 pipeline; fp32 PSUM accum"
    ):
        _emit_kernel(tc, xt_d, xt8_d, wqk8_d, wv_d, wo_d, bqk_d, bv_d,
                     outa_d, outb_d)
    nc.compile()
    return nc


def _emit_kernel(tc, xt_d, xt8_d, wqk8_d, wv_d, wo_d, bqk_d, bv_d,
                 outa_d, outb_d):
    nc = tc.nc
    scale = 1.0 / math.sqrt(HD)

    xt_r = xt_d.rearrange("(cb p) t -> p cb t", p=P)     # [128, 6, 2048]
    # fp8 operands for the QK projection, laid out for DoubleRow matmuls:
    # contraction chunk cs covers channels [cs*256, cs*256+256) as two
    # 128-partition k-tiles stacked on a free axis
    xt8_r = xt8_d.rearrange("(cs i p) t -> p cs i t", p=P, i=2)
    wqk8_r = wqk8_d.rearrange("(cs i p) o -> p cs i o", p=P, i=2)
    wv_r = wv_d.rearrange("(cb p) j -> p cb j", p=P)     # [128, 6, 384]
    wo_r = wo_d.rearrange("(jb p) o -> p jb o", p=P)     # [128, 3, 768]
    bqk_r = bqk_d.rearrange("(a p) -> p a", p=P)         # [128, 6]
    bv_r = bv_d.rearrange("(p a) -> p a", p=1)           # [1, 384]
    outa_r = outa_d.rearrange("(ob p) t -> p ob t", p=P)  # [128, 6, 2048]
    outb_r = outb_d.rearrange("(ob p) t -> p ob t", p=P)

    with (
        tc.tile_pool(name="persist", bufs=1) as persist,
        tc.tile_pool(name="stage", bufs=2) as stage,
        tc.tile_pool(name="attn", bufs=2) as attn,
        tc.tile_pool(name="ps512", bufs=2, space="PSUM") as ps512,
        tc.tile_pool(name="ps_s", bufs=2, space="PSUM") as ps_s,
        tc.tile_pool(name="ps_y", bufs=2, space="PSUM") as ps_y,
    ):
        xt = persist.tile([P, CS, T], BF16)       # x^T      24KB/partition
        xt8 = persist.tile([P, 3, 2, T], F8)      # x^T fp8  12KB
        wqk8 = persist.tile([P, 3, 2, 2 * J], F8)  # Wqk^T fp8 4.5KB
        wv = persist.tile([P, CS, J], BF16)       # Wv^T    4.5KB
        wo = persist.tile([P, JS, C], BF16)       # Wo^T    4.5KB
        qkvT = persist.tile([P, OQK, T], BF16)    # [q|k]^T  24KB
        vnat = persist.tile([P, TB, HL * VG], BF16)  # v natural 12.2KB
        yT = persist.tile([P, JS, T], BF16)       # y^T      12KB
        bsb = persist.tile([P, OQK], F32)
        bvrow = persist.tile([1, J], F32)
        brep = persist.tile([P, J], F32)

        # ---- input loads. HWDGE issues DMAs serially (~625ns each) and the
        # DMA engines run one transfer at a time (internally 16-way), so use
        # FEW large DMAs (>=512B contiguous runs where possible), ordered so
        # the first compute unit's data lands first.
        nc.sync.dma_start(wqk8, wqk8_r)
        nc.sync.dma_start(xt8[:, :, :, ts(0, 512)], xt8_r[:, :, :, ts(0, 512)])
        nc.sync.dma_start(xt8[:, :, :, ts(1, 512)], xt8_r[:, :, :, ts(1, 512)])
        nc.sync.dma_start(bsb, bqk_r)
        nc.sync.dma_start(bvrow, bv_r)
        nc.sync.dma_start(wv, wv_r)
        nc.sync.dma_start(xt[:, :, ts(0, 512)], xt_r[:, :, ts(0, 512)])
        nc.sync.dma_start(xt8[:, :, :, ts(2, 512)], xt8_r[:, :, :, ts(2, 512)])
        nc.sync.dma_start(xt[:, :, ts(1, 512)], xt_r[:, :, ts(1, 512)])
        nc.sync.dma_start(wo, wo_r)
        nc.sync.dma_start(xt8[:, :, :, ts(3, 512)], xt8_r[:, :, :, ts(3, 512)])
        nc.sync.dma_start(xt[:, :, ts(2, 512)], xt_r[:, :, ts(2, 512)])
        nc.sync.dma_start(xt[:, :, ts(3, 512)], xt_r[:, :, ts(3, 512)])

        # replicate v-bias across partitions; set the per-head ones columns
        nc.gpsimd.partition_broadcast(brep, bvrow)
        vnat4 = vnat[:, :, :].rearrange("p a (h e) -> p a h e", e=VG)
        nc.vector.memset(vnat4[:, :, :, HD : HD + 1], 1.0)
        brep3 = brep[:, :].rearrange("p (h e) -> p h e", e=HD)
        ones1f = persist.tile([1, HD], F32)
        nc.vector.memset(ones1f, 1.0)
        # 0/1 lower-triangle mask (keep q >= k): applied by a cheap DVE
        # multiply so the Pool queue holds only the normalize broadcasts and
        # masks never queue behind them
        trimask = persist.tile([P, P], BF16)
        nc.vector.memset(trimask, 1.0)
        nc.gpsimd.affine_select(
            out=trimask, in_=trimask,
            compare_op=mybir.AluOpType.is_ge,
            fill=0.0, base=0, channel_multiplier=-1,
            pattern=[[1, P]],
        )


        def emit_qkv(ob, tt):
            # qk^T[o, t] = sum_c Wqk^T[c, o] x^T[c, t] + b[o], via fp8
            # DoubleRow matmuls: each chunk contracts 256 channels at 0.5
            # cycles/column (4x fewer PE cycles than the bf16 version)
            pq = ps512.tile([P, 512], F32, tag="mm")
            for cs in range(3):
                nc.tensor.matmul(
                    pq,
                    wqk8[:, cs, :, ts(ob, P)],
                    xt8[:, cs, :, ts(tt, 512)],
                    start=(cs == 0),
                    stop=(cs == 2),
                    perf_mode=mybir.MatmulPerfMode.DoubleRow,
                )
            nc.vector.tensor_scalar_add(
                qkvT[:, ob, ts(tt, 512)], pq, bsb[:, ob : ob + 1]
            )

        def emit_vnat(tb):
            # v[t, j] = sum_c x^T[c, t] Wv^T[c, j]  (+ bias via brep)
            pv = ps512.tile([P, 512], F32, tag="mm")
            for cs in range(CS):
                nc.tensor.matmul(
                    pv[:, 0:J],
                    xt[:, cs, ts(tb, P)],
                    wv[:, cs, :],
                    start=(cs == 0),
                    stop=(cs == CS - 1),
                )
            nc.vector.tensor_add(
                out=vnat4[:, tb, :, 0:HD],
                in0=pv[:, 0:J].rearrange("p (h e) -> p h e", e=HD),
                in1=brep3,
            )

        # per-(output, tt) staging: 6 ob units copy into one tile, 1 DMA ships
        # it (coalesced transfer; keeps the HWDGE DMA count low)
        osb_tiles = {}

        def emit_outproj(tt, ob, js_list, okey, copy_eng):
            # part^T[o, t] = sum_{j in js_list} Wo^T[j, o] y^T[j, t]
            po = ps512.tile([P, 512], F32, tag="mm")
            for i, js in enumerate(js_list):
                nc.tensor.matmul(
                    po,
                    wo[:, js, ts(ob, P)],
                    yT[:, js, ts(tt, 512)],
                    start=(i == 0),
                    stop=(i == len(js_list) - 1),
                )
            out_r = outa_r if okey == "a" else outb_r
            if copy_eng == "dma":
                nc.sync.dma_start(out_r[:, ob, ts(tt, 512)], po)
                return
            if (okey, tt) not in osb_tiles:
                osb_tiles[(okey, tt)] = stage.tile(
                    [P, OUTB, 512], BF16, tag="ld", name=f"osb_{okey}_{tt}",
                    bufs=3,
                )
            osb = osb_tiles[(okey, tt)]
            if copy_eng == "act":
                nc.scalar.copy(osb[:, ob, :], po)
            else:
                nc.vector.tensor_copy(osb[:, ob, :], po)
            if okey == "b" and tt == 3:
                # endgame: ship per-ob so the last DMA is small
                if ob < 3:
                    if ob == 2:
                        nc.sync.dma_start(
                            outb_r[:, 0:3, ts(tt, 512)], osb[:, 0:3, :]
                        )
                else:
                    nc.sync.dma_start(
                        outb_r[:, ob, ts(tt, 512)], osb[:, ob, :]
                    )
            elif ob == OUTB - 1:
                nc.sync.dma_start(out_r[:, :, ts(tt, 512)], osb)

        # ---- phase 1: only what head 0's first half needs (QKV group 0 for
        # q-columns 0-1023, v for k-blocks 0-7); the rest becomes filler
        # inside head 0's ACT-paced stages
        emit_qkv(0, 0)
        emit_qkv(3, 0)
        emit_qkv(0, 1)
        emit_qkv(3, 1)
        for tb in range(8):
            emit_vnat(tb)

        # ---- filler work sprinkled into ACT-paced attention stages.
        # (head, hf, kb) -> list of thunks, run after the stage's PV.
        fillers = {}

        def add_filler(key, fn):
            fillers.setdefault(key, []).append(fn)

        # deferred phase-1 tail: QKV group 0 tt 2-3 + v-natural tb 8-15 land
        # inside head 0's early stages (needed from its hf1 half onward)
        for key, fn in (
            ((0, 0, 1), lambda: emit_qkv(0, 2)),
            ((0, 0, 2), lambda: emit_qkv(0, 3)),
            ((0, 0, 3), lambda: emit_vnat(8)),
            ((0, 0, 4), lambda: emit_qkv(3, 2)),
            ((0, 0, 5), lambda: emit_vnat(9)),
            ((0, 0, 6), lambda: emit_qkv(3, 3)),
            ((0, 0, 7), lambda: emit_vnat(10)),
            ((0, 1, 0), lambda: emit_vnat(11)),
            ((0, 1, 1), lambda: emit_vnat(12)),
            ((0, 1, 2), lambda: emit_vnat(13)),
            ((0, 1, 3), lambda: emit_vnat(14)),
            ((0, 1, 4), lambda: emit_vnat(15)),
        ):
            add_filler(key, fn)
        # QKV group 1 during heads 0-1, group 2 during heads 2-3
        for h, ob in ((0, 1), (1, 4), (2, 2), (3, 5)):
            for i, kb in enumerate((2, 5, 9, 13)):
                if h == 0:
                    kb = (6, 8, 10, 12)[i]
                add_filler((h, 1, kb), lambda ob=ob, tt=i: emit_qkv(ob, tt))
        # output projection partial A (heads 0-3, js 0..1): tt 0-1 during
        # head 3 (its qt muls land mid-head), tt 2-3 during head 4; one unit
        # per slot so the PE filler spreads across the ACT-paced stages
        oa_slots = {0: [(3, 1, 0), (3, 1, 1), (3, 1, 3), (3, 1, 4),
                        (3, 1, 6), (3, 1, 7)],
                    1: [(3, 1, 8), (3, 1, 10), (3, 1, 11), (3, 1, 12),
                        (3, 1, 14), (3, 1, 15)],
                    2: [(4, 0, 0), (4, 0, 1), (4, 0, 2), (4, 0, 3),
                        (4, 0, 5), (4, 0, 6)],
                    3: [(4, 0, 7), (4, 1, 0), (4, 1, 2), (4, 1, 4),
                        (4, 1, 6), (4, 1, 8)]}
        # output projection remainder B (js 2, heads 4-5) for tt 0-1 during
        # head 5; tt 2-3 go in the endgame drain
        obr_slots = {0: [(5, 1, 0), (5, 1, 1), (5, 1, 2), (5, 1, 3),
                         (5, 1, 4), (5, 1, 5)],
                     1: [(5, 1, 6), (5, 1, 7), (5, 1, 8), (5, 1, 9),
                         (5, 1, 10), (5, 1, 11)]}
        for slots, js_list, okey in ((oa_slots, [0, 1], "a"),
                                     (obr_slots, [2], "b")):
            for tt, keys in slots.items():
                for ob in range(OUTB):
                    add_filler(keys[ob], lambda tt=tt, ob=ob, js=js_list,
                               ok=okey: emit_outproj(tt, ob, js, ok, "dve"))

        # ---- the attention pipeline over all heads
        norm_q = []
        ya_tiles = {}

        def flush_norms():
            while norm_q:
                hl, qt, yu, rd, cb, w = norm_q.pop(0)
                p0 = (hl % 2) * HD
                if hl == HL - 1 and qt >= 2:
                    # endgame: PE is idle and the Pool->DVE chain is on the
                    # critical path, so broadcast the reciprocal with an
                    # fp32 ones-matmul on the PE instead
                    bcp = ps_s.tile([P, 1024], F32, tag="s", name="bcp")
                    nc.tensor.matmul(
                        bcp[0:HD, 0:w], ones1f, rd[:, 0:w],
                        start=True, stop=True,
                    )
                    bcb = bcp[0:HD, 0:w]
                else:
                    bcb = attn.tile([HD, 512], F32, tag="bc", name="bcb")
                    nc.gpsimd.partition_broadcast(bcb, rd)
                    bcb = bcb[:, 0:w]
                nc.vector.tensor_mul(
                    out=yT[p0 : p0 + HD, hl // 2, qt * 512 + cb : qt * 512 + cb + w],
                    in0=yu[:, 0:w],
                    in1=bcb,
                )

        pv_stash = []  # deferred single-qt PV thunks (ya-ring chain slack)

        def emit_pv_qt(hl, hf, kb, att, q0, lq, qt):
            c0 = max(0, qt * 512 - q0)
            c1 = min(lq, (qt + 1) * 512 - q0)
            o0 = q0 + c0 - qt * 512
            if (hl, qt) not in ya_tiles:
                ya_tiles[(hl, qt)] = ps_y.tile(
                    [P, 512], F32, tag="y", name=f"ya{hl}_{qt}"
                )
            ya = ya_tiles[(hl, qt)]
            nc.tensor.matmul(
                ya[0:VG, o0 : o0 + (c1 - c0)],
                vnat[:, kb, hl * VG : (hl + 1) * VG],
                att[:, c0:c1],
                start=(kb == 0),
                stop=(kb == 4 * qt + 3),
            )
            last_head_qt3 = hl == HL - 1 and qt == 3

            def emit_norm(cb, w):
                # pull y and the denominator straight out of PSUM so the ya
                # bank frees ~1us after the stop, independent of the
                # broadcast/multiply tail of the normalize chain
                rd = attn.tile([1, 512], F32, tag="rd", name="rd", bufs=3)
                nc.vector.reciprocal(rd[:, 0:w], ya[HD : HD + 1, cb : cb + w])
                yu = attn.tile([HD, 512], BF16, tag="yu", name="yu", bufs=3)
                nc.vector.tensor_copy(yu[:, 0:w], ya[0:HD, cb : cb + w])
                norm_q.append((hl, qt, yu, rd, cb, w))

            if last_head_qt3 and kb == 13:
                # columns 1536-1791 only attend to k-blocks <= 13, so their
                # slice of ya is final two stages before the qt stop: start
                # its normalize chain early to shorten the endgame tail
                emit_norm(0, 256)
            if kb == 4 * qt + 3:
                if last_head_qt3:
                    emit_norm(256, 256)
                else:
                    emit_norm(0, 512)

        def emit_pv(ent):
            hl, hf, kb, att, q0, lq = ent
            while pv_stash:
                pv_stash.pop(0)()
            qts = [qt for qt in (2 * hf, 2 * hf + 1) if kb <= 4 * qt + 3]
            if kb == 0 and len(qts) == 2:
                # defer the second qt one stage: its ya buffer is freed by a
                # normalize-multiply that is still in flight on DVE/Pool
                pv_stash.append(
                    lambda qt=qts[1]: emit_pv_qt(hl, hf, kb, att, q0, lq, qt)
                )
                qts = qts[:1]
            for qt in qts:
                emit_pv_qt(hl, hf, kb, att, q0, lq, qt)

        stages = [
            (hl, hf, kb)
            for hl in range(HL)
            for hf in (0, 1)
            for kb in range(8 if hf == 0 else 16)
        ]
        def emit_scores(sp, att_sl, kT, qT, kb, q0, lq, j0):
            # scores^T[k, q] into sp[:, j0:j0+lq]
            for j in range(0, lq, 512):
                f = min(512, lq - j)
                nc.tensor.matmul(
                    sp[:, j0 + j : j0 + j + f],
                    kT[:, ts(kb, P)],
                    qT[:, q0 + j : q0 + j + f],
                    start=True,
                    stop=True,
                )

        def emit_mask(att, kb, q0, j0):
            # diagonal block: zero out k > q entries
            if kb * P == q0:
                nc.vector.tensor_mul(
                    out=att[:, j0 : j0 + P],
                    in0=att[:, j0 : j0 + P],
                    in1=trimask,
                )

        # tail stages (lq <= 512) are emitted pairwise: both stages' scores
        # share one PSUM tile and a single exp, halving ACT dispatches there
        # (1,14)+(1,15) stay separate: the next head's second scores tile reuses
        # (1,15)'s PSUM slot, and a lone 128-col exp retires it much earlier
        # than the merged 384-col one (ACT has ~14us of slack under PE here)
        MERGE = {(0, 4): 5, (0, 6): 7, (1, 12): 13}
        follower_entries = {}
        pending = deque()
        for hl, hf, kb in stages:
            p0 = (hl % 2) * HD
            qT = qkvT[p0 : p0 + HD, hl // 2, :]
            kT = qkvT[p0 : p0 + HD, 3 + hl // 2, :]
            q0 = max(kb * P, hf * 1024)
            lq = (hf + 1) * 1024 - q0
            if (hl, hf, kb) in follower_entries:
                ent = follower_entries.pop((hl, hf, kb))
            elif (hf, kb) in MERGE:
                kb2 = MERGE[(hf, kb)]
                q02 = kb2 * P
                lq2 = (hf + 1) * 1024 - q02
                sp = ps_s.tile([P, 1024], F32, tag="s")
                att = attn.tile([P, 1024], BF16, tag="att", bufs=5)
                emit_scores(sp, att, kT, qT, kb, q0, lq, 0)
                emit_scores(sp, att, kT, qT, kb2, q02, lq2, lq)
                nc.scalar.activation(
                    att[:, : lq + lq2], sp[:, : lq + lq2],
                    mybir.ActivationFunctionType.Exp, scale=scale,
                )
                emit_mask(att, kb, q0, 0)
                emit_mask(att, kb2, q02, lq)
                ent = (hl, hf, kb, att[:, 0:lq], q0, lq)
                follower_entries[(hl, hf, kb2)] = (
                    hl, hf, kb2, att[:, lq : lq + lq2], q02, lq2
                )
            else:
                sp = ps_s.tile([P, 1024], F32, tag="s")
                att = attn.tile([P, 1024], BF16, tag="att", bufs=5)
                emit_scores(sp, att, kT, qT, kb, q0, lq, 0)
                nc.scalar.activation(
                    att[:, :lq], sp[:, :lq],
                    mybir.ActivationFunctionType.Exp, scale=scale,
                )
                emit_mask(att, kb, q0, 0)
                ent = (hl, hf, kb, att[:, 0:lq], q0, lq)
            flush_norms()
            if len(pending) >= 3:
                emit_pv(pending.popleft())
            pending.append(ent)
            for fn in fillers.get((hl, hf, kb), ()):
                fn()

        # ---- drain: final PVs, then tt2 remainder while the qt3 norm chain
        # completes on DVE/Pool, then the qt3-dependent tt3 remainder
        while len(pending) > 1:
            emit_pv(pending.popleft())
            flush_norms()
        emit_pv(pending.popleft())   # (5,1,15): stops qt3, queues its norm
        while pv_stash:
            pv_stash.pop(0)()
        for ob in range(OUTB):
            emit_outproj(2, ob, [2], "b", "act" if ob % 2 else "dve")
        flush_norms()                # qt3 normalize
        for ob in range(OUTB):
            emit_outproj(3, ob, [2], "b", "act" if ob % 2 else "dve")


_NC_CACHE = None
LAST_RESULTS = None


def _get_nc():
    global _NC_CACHE
    if _NC_CACHE is None:
        _NC_CACHE = _build_bass()
    return _NC_CACHE


def kernel(x, W_attn, b_attn, W_o, b_o):
    global LAST_RESULTS
    x = np.asarray(x, np.float32)
    W_attn = np.asarray(W_attn, np.float32)
    b_attn = np.asarray(b_attn, np.float32)
    W_o = np.asarray(W_o, np.float32)
    b_o = np.asarray(b_o, np.float32)
    bf = ml_dtypes.bfloat16

    B = x.shape[0]
    in_maps = []
    for core in range(8):
        b, hg = divmod(core, 2)
        sl = slice(hg * J, (hg + 1) * J)
        wq = W_attn[0:C][sl]
        wk = W_attn[C : 2 * C][hg * J : (hg + 1) * J]
        wvl = W_attn[2 * C : 3 * C][hg * J : (hg + 1) * J]
        f8 = ml_dtypes.float8_e4m3
        xtb = np.ascontiguousarray(x[b].T)
        in_maps.append({
            "xt": xtb.astype(bf),
            "xt8": xtb.astype(f8),
            "wqk8": np.ascontiguousarray(
                np.concatenate([wq, wk], 0).T).astype(f8),
            "wv": np.ascontiguousarray(wvl.T).astype(bf),
            "wo": np.ascontiguousarray(W_o[:, sl].T).astype(bf),
            "bqk": np.ascontiguousarray(
                np.concatenate([b_attn[sl], b_attn[C + hg * J : C + (hg + 1) * J]])
            ),
            "bv": np.ascontiguousarray(b_attn[2 * C + hg * J : 2 * C + (hg + 1) * J]),
        })

    nc = _get_nc()
    LAST_RESULTS = bass_utils.run_bass_kernel_spmd(
        nc, in_maps, core_ids=list(range(8)),
        trace=bool(int(os.environ.get("KERNEL_TRACE", "0"))),
    )
    out = np.empty((B, T, C), np.float32)
    for b in range(B):
        acc = None
        for r in (LAST_RESULTS.results[2 * b], LAST_RESULTS.results[2 * b + 1]):
            for key in ("outa", "outb"):
                part = np.asarray(r[key])
                acc = part.astype(np.float32) if acc is None else acc + part
        out[b] = acc.T + b_o
    return out



# revision 46
# speedup vs baseline: 1.0076x; 1.0076x over previous
"""Multi-head causal self-attention (B=4, T=2048, C=768, H=12) on 8 trn2 cores.

Sharding: core c handles batch b = c//2 and head-group hg = c%2 (6 heads each).
Host sums the output-projection partials per batch, transposes back, and adds
b_o. No cross-core collectives.

Final: single software-pipelined attention loop across all 6 heads
(PV matmuls run two stages behind their scores/exp, so the PE never waits on
ACT exp or Pool masking); QKV chunk groups and output-projection units are
sprinkled into ACT-paced attention stages as PE filler work; the output
projection is split into a heads-0-3 partial (computed during head 4 as
filler, shipped separately and summed on the host) and a heads-4-5 remainder
(tiny endgame tail); inputs arrive pre-transposed/pre-cast from the host
so there are zero PE transposes; the QK projection runs as fp8e4m3
DoubleRow matmuls (x and Wqk shipped in fp8, 256-channel contraction per
chunk at 0.5 cycles/column -- 4x fewer PE cycles than bf16, ~55k cycles
saved) while the V projection stays bf16 so v (which y is linear in) is
unquantized; V is built in natural [token, feature]
layout with per-head [64 v | 1 ones] groups giving PV its softmax denominator
row for free; normalization reads straight from PSUM (DVE reciprocal + Pool
partition-broadcast + DVE multiply).
"""

import math
import os
from collections import deque

import numpy as np
import ml_dtypes

import concourse.bass as bass
from concourse import bacc
import concourse.mybir as mybir
import concourse.tile as tile
from concourse import bass_utils
from concourse.bass import ts
from concourse.masks import make_identity

F32 = mybir.dt.float32
BF16 = mybir.dt.bfloat16
F8 = mybir.dt.float8e4

P = 128
T = 2048          # sequence length
C = 768           # embed dim
CS = C // P       # 6 contraction chunks
HL = 6            # heads per core
HD = 64           # head dim
J = HL * HD       # 384 local y-feature dim
JS = J // P       # 3
OQK = 2 * J // P  # 6 o-blocks of the local W_qk slice (q rows then k rows)
OUTB = C // P     # 6 output row blocks
TT = T // 512     # 4 column tiles of 512
TB = T // P       # 16 token blocks
VG = HD + 1       # 65: per-head v columns + ones column


def _build_bass():
    nc = bacc.Bacc("TRN2", target_bir_lowering=False, debug=False)
    xt_d = nc.dram_tensor("xt", [C, T], BF16, kind="ExternalInput").ap()
    xt8_d = nc.dram_tensor("xt8", [C, T], F8, kind="ExternalInput").ap()
    wqk8_d = nc.dram_tensor("wqk8", [C, 2 * J], F8, kind="ExternalInput").ap()
    wv_d = nc.dram_tensor("wv", [C, J], BF16, kind="ExternalInput").ap()
    wo_d = nc.dram_tensor("wo", [J, C], BF16, kind="ExternalInput").ap()
    bqk_d = nc.dram_tensor("bqk", [2 * J], F32, kind="ExternalInput").ap()
    bv_d = nc.dram_tensor("bv", [J], F32, kind="ExternalInput").ap()
    outa_d = nc.dram_tensor("outa", [C, T], BF16, kind="ExternalOutput").ap()
    outb_d = nc.dram_tensor("outb", [C, T], BF16, kind="ExternalOutput").ap()

    with tile.TileContext(nc) as tc, nc.allow_low_precision(
        reason="fp8 QK projection + bf16 pipeline; fp32 PSUM accum"
    ):
        _emit_kernel(tc, xt_d, xt8_d, wqk8_d, wv_d, wo_d, bqk_d, bv_d,
                     outa_d, outb_d)
    nc.compile()
    return nc


def _emit_kernel(tc, xt_d, xt8_d, wqk8_d, wv_d, wo_d, bqk_d, bv_d,
                 outa_d, outb_d):
    nc = tc.nc
    scale = 1.0 / math.sqrt(HD)

    xt_r = xt_d.rearrange("(cb p) t -> p cb t", p=P)     # [128, 6, 2048]
    # fp8 operands for the QK projection, laid out for DoubleRow matmuls:
    # contraction chunk cs covers channels [cs*256, cs*256+256) as two
    # 128-partition k-tiles stacked on a free axis
    xt8_r = xt8_d.rearrange("(cs i p) t -> p cs i t", p=P, i=2)
    wqk8_r = wqk8_d.rearrange("(cs i p) o -> p cs i o", p=P, i=2)
    wv_r = wv_d.rearrange("(cb p) j -> p cb j", p=P)     # [128, 6, 384]
    wo_r = wo_d.rearrange("(jb p) o -> p jb o", p=P)     # [128, 3, 768]
    bqk_r = bqk_d.rearrange("(a p) -> p a", p=P)         # [128, 6]
    bv_r = bv_d.rearrange("(p a) -> p a", p=1)           # [1, 384]
    outa_r = outa_d.rearrange("(ob p) t -> p ob t", p=P)  # [128, 6, 2048]
    outb_r = outb_d.rearrange("(ob p) t -> p ob t", p=P)

    with (
        tc.tile_pool(name="persist", bufs=1) as persist,
        tc.tile_pool(name="stage", bufs=2) as stage,
        tc.tile_pool(name="attn", bufs=2) as attn,
        tc.tile_pool(name="ps512", bufs=2, space="PSUM") as ps512,
        tc.tile_pool(name="ps_s", bufs=2, space="PSUM") as ps_s,
        tc.tile_pool(name="ps_y", bufs=2, space="PSUM") as ps_y,
    ):
        xt = persist.tile([P, CS, T], BF16)       # x^T      24KB/partition
        xt8 = persist.tile([P, 3, 2, T], F8)      # x^T fp8  12KB
        wqk8 = persist.tile([P, 3, 2, 2 * J], F8)  # Wqk^T fp8 4.5KB
        wv = persist.tile([P, CS, J], BF16)       # Wv^T    4.5KB
        wo = persist.tile([P, JS, C], BF16)       # Wo^T    4.5KB
        qkvT = persist.tile([P, OQK, T], BF16)    # [q|k]^T  24KB
        vnat = persist.tile([P, TB, HL * VG], BF16)  # v natural 12.2KB
        yT = persist.tile([P, JS, T], BF16)       # y^T      12KB
        bsb = persist.tile([P, OQK], F32)
        bvrow = persist.tile([1, J], F32)
        brep = persist.tile([P, J], F32)

        # ---- input loads. HWDGE issues DMAs serially (~625ns each) and the
        # DMA engines run one transfer at a time (internally 16-way), so use
        # FEW large DMAs (>=512B contiguous runs where possible), ordered so
        # the first compute unit's data lands first.
        nc.sync.dma_start(wqk8, wqk8_r)
        nc.sync.dma_start(xt8[:, :, :, ts(0, 512)], xt8_r[:, :, :, ts(0, 512)])
        nc.sync.dma_start(xt8[:, :, :, ts(1, 512)], xt8_r[:, :, :, ts(1, 512)])
        nc.sync.dma_start(bsb, bqk_r)
        nc.sync.dma_start(bvrow, bv_r)
        nc.sync.dma_start(wv, wv_r)
        nc.sync.dma_start(xt[:, :, ts(0, 512)], xt_r[:, :, ts(0, 512)])
        nc.sync.dma_start(xt8[:, :, :, ts(2, 512)], xt8_r[:, :, :, ts(2, 512)])
        nc.sync.dma_start(xt[:, :, ts(1, 512)], xt_r[:, :, ts(1, 512)])
        nc.sync.dma_start(wo, wo_r)
        nc.sync.dma_start(xt8[:, :, :, ts(3, 512)], xt8_r[:, :, :, ts(3, 512)])
        nc.sync.dma_start(xt[:, :, ts(2, 512)], xt_r[:, :, ts(2, 512)])
        nc.sync.dma_start(xt[:, :, ts(3, 512)], xt_r[:, :, ts(3, 512)])

        # replicate v-bias across partitions; set the per-head ones columns
        nc.gpsimd.partition_broadcast(brep, bvrow)
        vnat4 = vnat[:, :, :].rearrange("p a (h e) -> p a h e", e=VG)
        nc.vector.memset(vnat4[:, :, :, HD : HD + 1], 1.0)
        brep3 = brep[:, :].rearrange("p (h e) -> p h e", e=HD)
        ones1f = persist.tile([1, HD], F32)
        nc.vector.memset(ones1f, 1.0)
        # 0/1 lower-triangle mask (keep q >= k): applied by a cheap DVE
        # multiply so the Pool queue holds only the normalize broadcasts and
        # masks never queue behind them
        trimask = persist.tile([P, P], BF16)
        nc.vector.memset(trimask, 1.0)
        nc.gpsimd.affine_select(
            out=trimask, in_=trimask,
            compare_op=mybir.AluOpType.is_ge,
            fill=0.0, base=0, channel_multiplier=-1,
            pattern=[[1, P]],
        )


        def emit_qkv(ob, tt):
            # qk^T[o, t] = sum_c Wqk^T[c, o] x^T[c, t] + b[o], via fp8
            # DoubleRow matmuls: each chunk contracts 256 channels at 0.5
            # cycles/column (4x fewer PE cycles than the bf16 version)
            pq = ps512.tile([P, 512], F32, tag="mm")
            for cs in range(3):
                nc.tensor.matmul(
                    pq,
                    wqk8[:, cs, :, ts(ob, P)],
                    xt8[:, cs, :, ts(tt, 512)],
                    start=(cs == 0),
                    stop=(cs == 2),
                    perf_mode=mybir.MatmulPerfMode.DoubleRow,
                )
            nc.vector.tensor_scalar_add(
                qkvT[:, ob, ts(tt, 512)], pq, bsb[:, ob : ob + 1]
            )

        def emit_vnat(tb):
            # v[t, j] = sum_c x^T[c, t] Wv^T[c, j]  (+ bias via brep)
            pv = ps512.tile([P, 512], F32, tag="mm")
            for cs in range(CS):
                nc.tensor.matmul(
                    pv[:, 0:J],
                    xt[:, cs, ts(tb, P)],
                    wv[:, cs, :],
                    start=(cs == 0),
                    stop=(cs == CS - 1),
                )
            nc.vector.tensor_add(
                out=vnat4[:, tb, :, 0:HD],
                in0=pv[:, 0:J].rearrange("p (h e) -> p h e", e=HD),
                in1=brep3,
            )

        # per-(output, tt) staging: 6 ob units copy into one tile, 1 DMA ships
        # it (coalesced transfer; keeps the HWDGE DMA count low)
        osb_tiles = {}

        def emit_outproj(tt, ob, js_list, okey, copy_eng):
            # part^T[o, t] = sum_{j in js_list} Wo^T[j, o] y^T[j, t]
            po = ps512.tile([P, 512], F32, tag="mm")
            for i, js in enumerate(js_list):
                nc.tensor.matmul(
                    po,
                    wo[:, js, ts(ob, P)],
                    yT[:, js, ts(tt, 512)],
                    start=(i == 0),
                    stop=(i == len(js_list) - 1),
                )
            out_r = outa_r if okey == "a" else outb_r
            if copy_eng == "dma":
                nc.sync.dma_start(out_r[:, ob, ts(tt, 512)], po)
                return
            if (okey, tt) not in osb_tiles:
                osb_tiles[(okey, tt)] = stage.tile(
                    [P, OUTB, 512], BF16, tag="ld", name=f"osb_{okey}_{tt}",
                    bufs=3,
                )
            osb = osb_tiles[(okey, tt)]
            if copy_eng == "act":
                nc.scalar.copy(osb[:, ob, :], po)
            else:
                nc.vector.tensor_copy(osb[:, ob, :], po)
            if okey == "b" and tt == 3:
                # endgame: ship per-ob so the last DMA is small
                if ob < 3:
                    if ob == 2:
                        nc.sync.dma_start(
                            outb_r[:, 0:3, ts(tt, 512)], osb[:, 0:3, :]
                        )
                else:
                    nc.sync.dma_start(
                        outb_r[:, ob, ts(tt, 512)], osb[:, ob, :]
                    )
            elif ob == OUTB - 1:
                nc.sync.dma_start(out_r[:, :, ts(tt, 512)], osb)

        # ---- phase 1: only what head 0's first half needs (QKV group 0 for
        # q-columns 0-1023, v for k-blocks 0-7); the rest becomes filler
        # inside head 0's ACT-paced stages
        emit_qkv(0, 0)
        emit_qkv(3, 0)
        emit_qkv(0, 1)
        emit_qkv(3, 1)
        for tb in range(8):
            emit_vnat(tb)

        # ---- filler work sprinkled into ACT-paced attention stages.
        # (head, hf, kb) -> list of thunks, run after the stage's PV.
        fillers = {}

        def add_filler(key, fn):
            fillers.setdefault(key, []).append(fn)

        # deferred phase-1 tail: QKV group 0 tt 2-3 + v-natural tb 8-15 land
        # inside head 0's early stages (needed from its hf1 half onward)
        for key, fn in (
            ((0, 0, 1), lambda: emit_qkv(0, 2)),
            ((0, 0, 2), lambda: emit_qkv(0, 3)),
            ((0, 0, 3), lambda: emit_vnat(8)),
            ((0, 0, 4), lambda: emit_qkv(3, 2)),
            ((0, 0, 5), lambda: emit_vnat(9)),
            ((0, 0, 6), lambda: emit_qkv(3, 3)),
            ((0, 0, 7), lambda: emit_vnat(10)),
            ((0, 1, 0), lambda: emit_vnat(11)),
            ((0, 1, 1), lambda: emit_vnat(12)),
            ((0, 1, 2), lambda: emit_vnat(13)),
            ((0, 1, 3), lambda: emit_vnat(14)),
            ((0, 1, 4), lambda: emit_vnat(15)),
        ):
            add_filler(key, fn)
        # QKV group 1 during heads 0-1, group 2 during heads 2-3
        for h, ob in ((0, 1), (1, 4), (2, 2), (3, 5)):
            for i, kb in enumerate((2, 5, 9, 13)):
                if h == 0:
                    kb = (6, 8, 10, 12)[i]
                add_filler((h, 1, kb), lambda ob=ob, tt=i: emit_qkv(ob, tt))
        # output projection partial A (heads 0-3, js 0..1): tt 0-1 during
        # head 3 (its qt muls land mid-head), tt 2-3 during head 4; one unit
        # per slot so the PE filler spreads across the ACT-paced stages
        oa_slots = {0: [(3, 1, 0), (3, 1, 1), (3, 1, 3), (3, 1, 4),
                        (3, 1, 6), (3, 1, 7)],
                    1: [(3, 1, 8), (3, 1, 10), (3, 1, 11), (3, 1, 12),
                        (3, 1, 14), (3, 1, 15)],
                    2: [(4, 0, 0), (4, 0, 1), (4, 0, 2), (4, 0, 3),
                        (4, 0, 5), (4, 0, 6)],
                    3: [(4, 0, 7), (4, 1, 0), (4, 1, 2), (4, 1, 4),
                        (4, 1, 6), (4, 1, 8)]}
        # output projection remainder B (js 2, heads 4-5) for tt 0-1 during
        # head 5; tt 2-3 go in the endgame drain
        obr_slots = {0: [(5, 1, 0), (5, 1, 1), (5, 1, 2), (5, 1, 3),
                         (5, 1, 4), (5, 1, 5)],
                     1: [(5, 1, 6), (5, 1, 7), (5, 1, 8), (5, 1, 9),
                         (5, 1, 10), (5, 1, 11)]}
        for slots, js_list, okey in ((oa_slots, [0, 1], "a"),
                                     (obr_slots, [2], "b")):
            for tt, keys in slots.items():
                for ob in range(OUTB):
                    add_filler(keys[ob], lambda tt=tt, ob=ob, js=js_list,
                               ok=okey: emit_outproj(tt, ob, js, ok, "dve"))

        # ---- the attention pipeline over all heads
        norm_q = []
        ya_tiles = {}

        def flush_norms():
            while norm_q:
                hl, qt, yu, rd, cb, w = norm_q.pop(0)
                p0 = (hl % 2) * HD
                if hl == HL - 1 and qt >= 2:
                    # endgame: PE is idle and the Pool->DVE chain is on the
                    # critical path, so broadcast the reciprocal with an
                    # fp32 ones-matmul on the PE instead
                    bcp = ps_s.tile([P, 1024], F32, tag="s", name="bcp")
                    nc.tensor.matmul(
                        bcp[0:HD, 0:w], ones1f, rd[:, 0:w],
                        start=True, stop=True,
                    )
                    bcb = bcp[0:HD, 0:w]
                else:
                    bcb = attn.tile([HD, 512], F32, tag="bc", name="bcb")
                    nc.gpsimd.partition_broadcast(bcb, rd)
                    bcb = bcb[:, 0:w]
                nc.vector.tensor_mul(
                    out=yT[p0 : p0 + HD, hl // 2, qt * 512 + cb : qt * 512 + cb + w],
                    in0=yu[:, 0:w],
                    in1=bcb,
                )

        pv_stash = []  # deferred single-qt PV thunks (ya-ring chain slack)

        def emit_pv_qt(hl, hf, kb, att, q0, lq, qt):
            c0 = max(0, qt * 512 - q0)
            c1 = min(lq, (qt + 1) * 512 - q0)
            o0 = q0 + c0 - qt * 512
            if (hl, qt) not in ya_tiles:
                ya_tiles[(hl, qt)] = ps_y.tile(
                    [P, 512], F32, tag="y", name=f"ya{hl}_{qt}"
                )
            ya = ya_tiles[(hl, qt)]
            nc.tensor.matmul(
                ya[0:VG, o0 : o0 + (c1 - c0)],
                vnat[:, kb, hl * VG : (hl + 1) * VG],
                att[:, c0:c1],
                start=(kb == 0),
                stop=(kb == 4 * qt + 3),
            )
            last_head_qt3 = hl == HL - 1 and qt == 3

            def emit_norm(cb, w):
                # pull y and the denominator straight out of PSUM so the ya
                # bank frees ~1us after the stop, independent of the
                # broadcast/multiply tail of the normalize chain
                rd = attn.tile([1, 512], F32, tag="rd", name="rd", bufs=3)
                nc.vector.reciprocal(rd[:, 0:w], ya[HD : HD + 1, cb : cb + w])
                yu = attn.tile([HD, 512], BF16, tag="yu", name="yu", bufs=3)
                nc.vector.tensor_copy(yu[:, 0:w], ya[0:HD, cb : cb + w])
                norm_q.append((hl, qt, yu, rd, cb, w))

            if last_head_qt3 and kb == 13:
                # columns 1536-1791 only attend to k-blocks <= 13, so their
                # slice of ya is final two stages before the qt stop: start
                # its normalize chain early to shorten the endgame tail
                emit_norm(0, 256)
            if kb == 4 * qt + 3:
                if last_head_qt3:
                    emit_norm(256, 256)
                else:
                    emit_norm(0, 512)

        def emit_pv(ent):
            hl, hf, kb, att, q0, lq = ent
            while pv_stash:
                pv_stash.pop(0)()
            qts = [qt for qt in (2 * hf, 2 * hf + 1) if kb <= 4 * qt + 3]
            if kb == 0 and len(qts) == 2:
                # defer the second qt one stage: its ya buffer is freed by a
                # normalize-multiply that is still in flight on DVE/Pool
                pv_stash.append(
                    lambda qt=qts[1]: emit_pv_qt(hl, hf, kb, att, q0, lq, qt)
                )
                qts = qts[:1]
            for qt in qts:
                emit_pv_qt(hl, hf, kb, att, q0, lq, qt)

        stages = [
            (hl, hf, kb)
            for hl in range(HL)
            for hf in (0, 1)
            for kb in range(8 if hf == 0 else 16)
        ]
        def emit_scores(sp, att_sl, kT, qT, kb, q0, lq, j0):
            # scores^T[k, q] into sp[:, j0:j0+lq]
            for j in range(0, lq, 512):
                f = min(512, lq - j)
                nc.tensor.matmul(
                    sp[:, j0 + j : j0 + j + f],
                    kT[:, ts(kb, P)],
                    qT[:, q0 + j : q0 + j + f],
                    start=True,
                    stop=True,
                )

        def emit_mask(att, kb, q0, j0):
            # diagonal block: zero out k > q entries
            if kb * P == q0:
                nc.vector.tensor_mul(
                    out=att[:, j0 : j0 + P],
                    in0=att[:, j0 : j0 + P],
                    in1=trimask,
                )

        # tail stages (lq <= 512) are emitted pairwise: both stages' scores
        # share one PSUM tile and a single exp, halving ACT dispatches there
        MERGE = {(0, 4): 5, (0, 6): 7, (1, 12): 13, (1, 14): 15}
        follower_entries = {}
        pending = deque()
        for hl, hf, kb in stages:
            p0 = (hl % 2) * HD
            qT = qkvT[p0 : p0 + HD, hl // 2, :]
            kT = qkvT[p0 : p0 + HD, 3 + hl // 2, :]
            q0 = max(kb * P, hf * 1024)
            lq = (hf + 1) * 1024 - q0
            if (hl, hf, kb) in follower_entries:
                ent = follower_entries.pop((hl, hf, kb))
            elif (hf, kb) in MERGE:
                kb2 = MERGE[(hf, kb)]
                q02 = kb2 * P
                lq2 = (hf + 1) * 1024 - q02
                sp = ps_s.tile([P, 1024], F32, tag="s")
                att = attn.tile([P, 1024], BF16, tag="att", bufs=5)
                emit_scores(sp, att, kT, qT, kb, q0, lq, 0)
                emit_scores(sp, att, kT, qT, kb2, q02, lq2, lq)
                nc.scalar.activation(
                    att[:, : lq + lq2], sp[:, : lq + lq2],
                    mybir.ActivationFunctionType.Exp, scale=scale,
                )
                emit_mask(att, kb, q0, 0)
                emit_mask(att, kb2, q02, lq)
                ent = (hl, hf, kb, att[:, 0:lq], q0, lq)
                follower_entries[(hl, hf, kb2)] = (
                    hl, hf, kb2, att[:, lq : lq + lq2], q02, lq2
                )
            else:
                sp = ps_s.tile([P, 1024], F32, tag="s")
                att = attn.tile([P, 1024], BF16, tag="att", bufs=5)
                emit_scores(sp, att, kT, qT, kb, q0, lq, 0)
                nc.scalar.activation(
                    att[:, :lq], sp[:, :lq],
                    mybir.ActivationFunctionType.Exp, scale=scale,
                )
                emit_mask(att, kb, q0, 0)
                ent = (hl, hf, kb, att[:, 0:lq], q0, lq)
            flush_norms()
            if len(pending) >= 2:
                emit_pv(pending.popleft())
            pending.append(ent)
            for fn in fillers.get((hl, hf, kb), ()):
                fn()

        # ---- drain: final PVs, then tt2 remainder while the qt3 norm chain
        # completes on DVE/Pool, then the qt3-dependent tt3 remainder
        while len(pending) > 1:
            emit_pv(pending.popleft())
            flush_norms()
        emit_pv(pending.popleft())   # (5,1,15): stops qt3, queues its norm
        while pv_stash:
            pv_stash.pop(0)()
        for ob in range(OUTB):
            emit_outproj(2, ob, [2], "b", "act" if ob % 2 else "dve")
        flush_norms()                # qt3 normalize
        for ob in range(OUTB):
            emit_outproj(3, ob, [2], "b", "act" if ob % 2 else "dve")


_NC_CACHE = None
LAST_RESULTS = None


def _get_nc():
    global _NC_CACHE
    if _NC_CACHE is None:
        _NC_CACHE = _build_bass()
    return _NC_CACHE


def kernel(x, W_attn, b_attn, W_o, b_o):
    global LAST_RESULTS
    x = np.asarray(x, np.float32)
    W_attn = np.asarray(W_attn, np.float32)
    b_attn = np.asarray(b_attn, np.float32)
    W_o = np.asarray(W_o, np.float32)
    b_o = np.asarray(b_o, np.float32)
    bf = ml_dtypes.bfloat16

    B = x.shape[0]
    in_maps = []
    for core in range(8):
        b, hg = divmod(core, 2)
        sl = slice(hg * J, (hg + 1) * J)
        wq = W_attn[0:C][sl]
        wk = W_attn[C : 2 * C][hg * J : (hg + 1) * J]
        wvl = W_attn[2 * C : 3 * C][hg * J : (hg + 1) * J]
        f8 = ml_dtypes.float8_e4m3
        xtb = np.ascontiguousarray(x[b].T)
        in_maps.append({
            "xt": xtb.astype(bf),
            "xt8": xtb.astype(f8),
            "wqk8": np.ascontiguousarray(
                np.concatenate([wq, wk], 0).T).astype(f8),
            "wv": np.ascontiguousarray(wvl.T).astype(bf),
            "wo": np.ascontiguousarray(W_o[:, sl].T).astype(bf),
            "bqk": np.ascontiguousarray(
                np.concatenate([b_attn[sl], b_attn[C + hg * J : C + (hg + 1) * J]])
            ),
            "bv": np.ascontiguousarray(b_attn[2 * C + hg * J : 2 * C + (hg + 1) * J]),
        })

    nc = _get_nc()
    LAST_RESULTS = bass_utils.run_bass_kernel_spmd(
        nc, in_maps, core_ids=list(range(8)),
        trace=bool(int(os.environ.get("KERNEL_TRACE", "0"))),
    )
    out = np.empty((B, T, C), np.float32)
    for b in range(B):
        acc = None
        for r in (LAST_RESULTS.results[2 * b], LAST_RESULTS.results[2 * b + 1]):
            for key in ("outa", "outb"):
                part = np.asarray(r[key])
                acc = part.astype(np.float32) if acc is None else acc + part
        out[b] = acc.T + b_o
    return out



# revision 48
# speedup vs baseline: 1.0403x; 1.0325x over previous
"""Multi-head causal self-attention (B=4, T=2048, C=768, H=12) on 8 trn2 cores.

Sharding: core c handles batch b = c//2 and head-group hg = c%2 (6 heads each).
Host sums the output-projection partials per batch, transposes back, and adds
b_o. No cross-core collectives.

Final: single software-pipelined attention loop across all 6 heads
(PV matmuls run two stages behind their scores/exp, so the PE never waits on
ACT exp or Pool masking); QKV chunk groups and output-projection units are
sprinkled into ACT-paced attention stages as PE filler work; the output
projection is split into a heads-0-3 partial (computed during head 4 as
filler, shipped separately and summed on the host) and a heads-4-5 remainder
(tiny endgame tail); inputs arrive pre-transposed/pre-cast from the host
so there are zero PE transposes; the QK projection runs as fp8e4m3
DoubleRow matmuls (x and Wqk shipped in fp8, 256-channel contraction per
chunk at 0.5 cycles/column -- 4x fewer PE cycles than bf16, ~55k cycles
saved) while the V projection stays bf16 so v (which y is linear in) is
unquantized; V is built in natural [token, feature]
layout with per-head [64 v | 1 ones] groups giving PV its softmax denominator
row for free; normalization reads straight from PSUM (DVE reciprocal + Pool
partition-broadcast + DVE multiply).
"""

import math
import os
from collections import deque

import numpy as np
import ml_dtypes

import concourse.bass as bass
from concourse import bacc
import concourse.mybir as mybir
import concourse.tile as tile
from concourse import bass_utils
from concourse.bass import ts
from concourse.masks import make_identity

F32 = mybir.dt.float32
BF16 = mybir.dt.bfloat16
F8 = mybir.dt.float8e4

P = 128
T = 2048          # sequence length
C = 768           # embed dim
CS = C // P       # 6 contraction chunks
HL = 6            # heads per core
HD = 64           # head dim
J = HL * HD       # 384 local y-feature dim
JS = J // P       # 3
OQK = 2 * J // P  # 6 o-blocks of the local W_qk slice (q rows then k rows)
OUTB = C // P     # 6 output row blocks
TT = T // 512     # 4 column tiles of 512
TB = T // P       # 16 token blocks
VG = HD + 1       # 65: per-head v columns + ones column


def _build_bass():
    nc = bacc.Bacc("TRN2", target_bir_lowering=False, debug=False)
    xt_d = nc.dram_tensor("xt", [C, T], BF16, kind="ExternalInput").ap()
    xt8_d = nc.dram_tensor("xt8", [C, T], F8, kind="ExternalInput").ap()
    wqk8_d = nc.dram_tensor("wqk8", [C, 2 * J], F8, kind="ExternalInput").ap()
    wv_d = nc.dram_tensor("wv", [C, J], BF16, kind="ExternalInput").ap()
    wo_d = nc.dram_tensor("wo", [J, C], BF16, kind="ExternalInput").ap()
    bqk_d = nc.dram_tensor("bqk", [2 * J], F32, kind="ExternalInput").ap()
    bv_d = nc.dram_tensor("bv", [J], F32, kind="ExternalInput").ap()
    outa_d = nc.dram_tensor("outa", [C, T], BF16, kind="ExternalOutput").ap()
    outb_d = nc.dram_tensor("outb", [C, T], BF16, kind="ExternalOutput").ap()

    with tile.TileContext(nc) as tc, nc.allow_low_precision(
        reason="fp8 QK projection + bf16 pipeline; fp32 PSUM accum"
    ):
        _emit_kernel(tc, xt_d, xt8_d, wqk8_d, wv_d, wo_d, bqk_d, bv_d,
                     outa_d, outb_d)
    nc.compile()
    return nc


def _emit_kernel(tc, xt_d, xt8_d, wqk8_d, wv_d, wo_d, bqk_d, bv_d,
                 outa_d, outb_d):
    nc = tc.nc
    scale = 1.0 / math.sqrt(HD)

    xt_r = xt_d.rearrange("(cb p) t -> p cb t", p=P)     # [128, 6, 2048]
    # fp8 operands for the QK projection, laid out for DoubleRow matmuls:
    # contraction chunk cs covers channels [cs*256, cs*256+256) as two
    # 128-partition k-tiles stacked on a free axis
    xt8_r = xt8_d.rearrange("(cs i p) t -> p cs i t", p=P, i=2)
    wqk8_r = wqk8_d.rearrange("(cs i p) o -> p cs i o", p=P, i=2)
    wv_r = wv_d.rearrange("(cb p) j -> p cb j", p=P)     # [128, 6, 384]
    wo_r = wo_d.rearrange("(jb p) o -> p jb o", p=P)     # [128, 3, 768]
    bqk_r = bqk_d.rearrange("(a p) -> p a", p=P)         # [128, 6]
    bv_r = bv_d.rearrange("(p a) -> p a", p=1)           # [1, 384]
    outa_r = outa_d.rearrange("(ob p) t -> p ob t", p=P)  # [128, 6, 2048]
    outb_r = outb_d.rearrange("(ob p) t -> p ob t", p=P)

    with (
        tc.tile_pool(name="persist", bufs=1) as persist,
        tc.tile_pool(name="stage", bufs=2) as stage,
        tc.tile_pool(name="attn", bufs=2) as attn,
        tc.tile_pool(name="ps512", bufs=2, space="PSUM") as ps512,
        tc.tile_pool(name="ps_s", bufs=2, space="PSUM") as ps_s,
        tc.tile_pool(name="ps_y", bufs=2, space="PSUM") as ps_y,
    ):
        xt = persist.tile([P, CS, T], BF16)       # x^T      24KB/partition
        xt8 = persist.tile([P, 3, 2, T], F8)      # x^T fp8  12KB
        wqk8 = persist.tile([P, 3, 2, 2 * J], F8)  # Wqk^T fp8 4.5KB
        wv = persist.tile([P, CS, J], BF16)       # Wv^T    4.5KB
        wo = persist.tile([P, JS, C], BF16)       # Wo^T    4.5KB
        qkvT = persist.tile([P, OQK, T], F8)      # [q|k]^T fp8 12KB
        # folded q/k for fp8 DoubleRow scores: head hl lives at partition
        # base 32*(hl%3) (the PE only accepts stationary bases 0/32/64),
        # free slot hl//3, with the two head-dim halves adjacent on a free
        # axis: qkf[base:base+32, q|k, slot, half, T]
        qkf = persist.tile([P, 2, 2, 2, T], F8)   # 16KB
        vnat = persist.tile([P, TB, HL * VG], BF16)  # v natural 12.2KB
        yT = persist.tile([P, JS, T], BF16)       # y^T      12KB
        bsb = persist.tile([P, OQK], F32)
        bvrow = persist.tile([1, J], F32)
        brep = persist.tile([P, J], F32)

        # ---- input loads. HWDGE issues DMAs serially (~625ns each) and the
        # DMA engines run one transfer at a time (internally 16-way), so use
        # FEW large DMAs (>=512B contiguous runs where possible), ordered so
        # the first compute unit's data lands first.
        nc.sync.dma_start(wqk8, wqk8_r)
        nc.sync.dma_start(xt8[:, :, :, ts(0, 512)], xt8_r[:, :, :, ts(0, 512)])
        nc.sync.dma_start(xt8[:, :, :, ts(1, 512)], xt8_r[:, :, :, ts(1, 512)])
        nc.sync.dma_start(bsb, bqk_r)
        nc.sync.dma_start(bvrow, bv_r)
        nc.sync.dma_start(wv, wv_r)
        nc.sync.dma_start(xt[:, :, ts(0, 512)], xt_r[:, :, ts(0, 512)])
        nc.sync.dma_start(xt8[:, :, :, ts(2, 512)], xt8_r[:, :, :, ts(2, 512)])
        nc.sync.dma_start(xt[:, :, ts(1, 512)], xt_r[:, :, ts(1, 512)])
        nc.sync.dma_start(wo, wo_r)
        nc.sync.dma_start(xt8[:, :, :, ts(3, 512)], xt8_r[:, :, :, ts(3, 512)])
        nc.sync.dma_start(xt[:, :, ts(2, 512)], xt_r[:, :, ts(2, 512)])
        nc.sync.dma_start(xt[:, :, ts(3, 512)], xt_r[:, :, ts(3, 512)])

        # replicate v-bias across partitions; set the per-head ones columns
        nc.gpsimd.partition_broadcast(brep, bvrow)
        vnat4 = vnat[:, :, :].rearrange("p a (h e) -> p a h e", e=VG)
        nc.vector.memset(vnat4[:, :, :, HD : HD + 1], 1.0)
        brep3 = brep[:, :].rearrange("p (h e) -> p h e", e=HD)
        ones1f = persist.tile([1, HD], F32)
        nc.vector.memset(ones1f, 1.0)
        # 0/1 lower-triangle mask (keep q >= k): applied by a cheap DVE
        # multiply so the Pool queue holds only the normalize broadcasts and
        # masks never queue behind them
        trimask = persist.tile([P, P], BF16)
        nc.vector.memset(trimask, 1.0)
        nc.gpsimd.affine_select(
            out=trimask, in_=trimask,
            compare_op=mybir.AluOpType.is_ge,
            fill=0.0, base=0, channel_multiplier=-1,
            pattern=[[1, P]],
        )


        def emit_qkv(ob, tt):
            # qk^T[o, t] = sum_c Wqk^T[c, o] x^T[c, t] + b[o], via fp8
            # DoubleRow matmuls: each chunk contracts 256 channels at 0.5
            # cycles/column (4x fewer PE cycles than the bf16 version)
            pq = ps512.tile([P, 512], F32, tag="mm")
            for cs in range(3):
                nc.tensor.matmul(
                    pq,
                    wqk8[:, cs, :, ts(ob, P)],
                    xt8[:, cs, :, ts(tt, 512)],
                    start=(cs == 0),
                    stop=(cs == 2),
                    perf_mode=mybir.MatmulPerfMode.DoubleRow,
                )
            nc.vector.tensor_scalar_add(
                qkvT[:, ob, ts(tt, 512)], pq, bsb[:, ob : ob + 1]
            )

        def emit_vnat(tb):
            # v[t, j] = sum_c x^T[c, t] Wv^T[c, j]  (+ bias via brep)
            pv = ps512.tile([P, 512], F32, tag="mm")
            for cs in range(CS):
                nc.tensor.matmul(
                    pv[:, 0:J],
                    xt[:, cs, ts(tb, P)],
                    wv[:, cs, :],
                    start=(cs == 0),
                    stop=(cs == CS - 1),
                )
            nc.vector.tensor_add(
                out=vnat4[:, tb, :, 0:HD],
                in0=pv[:, 0:J].rearrange("p (h e) -> p h e", e=HD),
                in1=brep3,
            )

        # per-(output, tt) staging: 6 ob units copy into one tile, 1 DMA ships
        # it (coalesced transfer; keeps the HWDGE DMA count low)
        osb_tiles = {}

        def emit_outproj(tt, ob, js_list, okey, copy_eng):
            # part^T[o, t] = sum_{j in js_list} Wo^T[j, o] y^T[j, t]
            po = ps512.tile([P, 512], F32, tag="mm")
            for i, js in enumerate(js_list):
                nc.tensor.matmul(
                    po,
                    wo[:, js, ts(ob, P)],
                    yT[:, js, ts(tt, 512)],
                    start=(i == 0),
                    stop=(i == len(js_list) - 1),
                )
            out_r = outa_r if okey == "a" else outb_r
            if copy_eng == "dma":
                nc.sync.dma_start(out_r[:, ob, ts(tt, 512)], po)
                return
            if (okey, tt) not in osb_tiles:
                osb_tiles[(okey, tt)] = stage.tile(
                    [P, OUTB, 512], BF16, tag="ld", name=f"osb_{okey}_{tt}",
                    bufs=3,
                )
            osb = osb_tiles[(okey, tt)]
            if copy_eng == "act":
                nc.scalar.copy(osb[:, ob, :], po)
            else:
                nc.vector.tensor_copy(osb[:, ob, :], po)
            if okey == "b" and tt == 3:
                # endgame: ship per-ob so the last DMA is small
                if ob < 3:
                    if ob == 2:
                        nc.sync.dma_start(
                            outb_r[:, 0:3, ts(tt, 512)], osb[:, 0:3, :]
                        )
                else:
                    nc.sync.dma_start(
                        outb_r[:, ob, ts(tt, 512)], osb[:, ob, :]
                    )
            elif ob == OUTB - 1:
                nc.sync.dma_start(out_r[:, :, ts(tt, 512)], osb)

        # ---- phase 1: only what head 0's first half needs (QKV group 0 for
        # q-columns 0-1023, v for k-blocks 0-7); the rest becomes filler
        # inside head 0's ACT-paced stages
        emit_qkv(0, 0)
        emit_qkv(3, 0)
        emit_qkv(0, 1)
        emit_qkv(3, 1)
        for tb in range(8):
            emit_vnat(tb)

        def fold_head(hl, lo=0, hi=T):
            # rebuild head hl's q/k as [32, 2(half), T] via partition-
            # remapping SBUF->SBUF DMAs (~180ns transfers).  IMPORTANT: a
            # fold must never be emitted before every qkvT column it reads
            # has been written -- an early fold captures stale SBUF (WAR
            # ordering makes this "legal"), which poisons run 1 only.
            a, s, o = 32 * (hl % 3), hl // 3, hl % 2
            for qk, ob in ((0, hl // 2), (1, 3 + hl // 2)):
                for i in (0, 1):
                    nc.sync.dma_start(
                        qkf[a : a + 32, qk, s, i, lo:hi],
                        qkvT[64 * o + 32 * i : 64 * o + 32 * i + 32, ob,
                             lo:hi],
                    )

        # heads 0/1: columns 0-1023 exist after the preamble; columns
        # 1024-2047 only after the head-0 fillers finish qkv(0,2/3)+(3,2/3),
        # so those halves fold inside head 0's stages (consumers are in hf1)
        fold_head(0, 0, 1024)
        fold_head(1, 0, 1024)

        # ---- filler work sprinkled into ACT-paced attention stages.
        # (head, hf, kb) -> list of thunks, run after the stage's PV.
        fillers = {}

        def add_filler(key, fn):
            fillers.setdefault(key, []).append(fn)

        # deferred phase-1 tail: QKV group 0 tt 2-3 + v-natural tb 8-15 land
        # inside head 0's early stages (needed from its hf1 half onward)
        for key, fn in (
            ((0, 0, 1), lambda: emit_qkv(0, 2)),
            ((0, 0, 2), lambda: emit_qkv(0, 3)),
            ((0, 0, 3), lambda: emit_vnat(8)),
            ((0, 0, 4), lambda: emit_qkv(3, 2)),
            ((0, 0, 5), lambda: emit_vnat(9)),
            ((0, 0, 6), lambda: emit_qkv(3, 3)),
            ((0, 0, 7), lambda: emit_vnat(10)),
            ((0, 1, 0), lambda: emit_vnat(11)),
            ((0, 1, 1), lambda: emit_vnat(12)),
            ((0, 1, 2), lambda: emit_vnat(13)),
            ((0, 1, 3), lambda: emit_vnat(14)),
            ((0, 1, 4), lambda: emit_vnat(15)),
        ):
            add_filler(key, fn)
        # QKV group 1 during heads 0-1, group 2 during heads 2-3
        for h, ob, kbs in ((0, 1, (6, 8, 10, 12)), (1, 4, (2, 5, 9, 13)),
                           (2, 2, (2, 5, 9, 13)), (2, 5, (3, 6, 10, 14))):
            for i, kb in enumerate(kbs):
                add_filler((h, 1, kb), lambda ob=ob, tt=i: emit_qkv(ob, tt))
        # folds become available as soon as the last tt of each head's q and
        # k blocks has been copied into qkvT
        add_filler((0, 0, 6), lambda: fold_head(0, 1024, 2048))
        add_filler((0, 0, 7), lambda: fold_head(1, 1024, 2048))
        add_filler((1, 1, 14), lambda: fold_head(2))
        add_filler((1, 1, 15), lambda: fold_head(3))
        add_filler((2, 1, 15), lambda: fold_head(4))
        add_filler((3, 0, 0), lambda: fold_head(5))
        # output projection partial A (heads 0-3, js 0..1): tt 0-1 during
        # head 3 (its qt muls land mid-head), tt 2-3 during head 4; one unit
        # per slot so the PE filler spreads across the ACT-paced stages
        oa_slots = {0: [(3, 1, 0), (3, 1, 1), (3, 1, 3), (3, 1, 4),
                        (3, 1, 6), (3, 1, 7)],
                    1: [(3, 1, 8), (3, 1, 10), (3, 1, 11), (3, 1, 12),
                        (3, 1, 14), (3, 1, 15)],
                    2: [(4, 0, 0), (4, 0, 1), (4, 0, 2), (4, 0, 3),
                        (4, 0, 5), (4, 0, 6)],
                    3: [(4, 0, 7), (4, 1, 0), (4, 1, 2), (4, 1, 4),
                        (4, 1, 6), (4, 1, 8)]}
        # output projection remainder B (js 2, heads 4-5) for tt 0-1 during
        # head 5; tt 2-3 go in the endgame drain
        obr_slots = {0: [(5, 1, 0), (5, 1, 1), (5, 1, 2), (5, 1, 3),
                         (5, 1, 4), (5, 1, 5)],
                     1: [(5, 1, 6), (5, 1, 7), (5, 1, 8), (5, 1, 9),
                         (5, 1, 10), (5, 1, 11)]}
        for slots, js_list, okey in ((oa_slots, [0, 1], "a"),
                                     (obr_slots, [2], "b")):
            for tt, keys in slots.items():
                for ob in range(OUTB):
                    add_filler(keys[ob], lambda tt=tt, ob=ob, js=js_list,
                               ok=okey: emit_outproj(tt, ob, js, ok, "dve"))

        # ---- the attention pipeline over all heads
        norm_q = []
        ya_tiles = {}

        def flush_norms():
            while norm_q:
                hl, qt, yu, rd, cb, w = norm_q.pop(0)
                p0 = (hl % 2) * HD
                if hl == HL - 1 and qt >= 2:
                    # endgame: PE is idle and the Pool->DVE chain is on the
                    # critical path, so broadcast the reciprocal with an
                    # fp32 ones-matmul on the PE instead
                    bcp = ps_s.tile([P, 1024], F32, tag="s", name="bcp")
                    nc.tensor.matmul(
                        bcp[0:HD, 0:w], ones1f, rd[:, 0:w],
                        start=True, stop=True,
                    )
                    bcb = bcp[0:HD, 0:w]
                else:
                    bcb = attn.tile([HD, 512], F32, tag="bc", name="bcb")
                    nc.gpsimd.partition_broadcast(bcb, rd)
                    bcb = bcb[:, 0:w]
                nc.vector.tensor_mul(
                    out=yT[p0 : p0 + HD, hl // 2, qt * 512 + cb : qt * 512 + cb + w],
                    in0=yu[:, 0:w],
                    in1=bcb,
                )

        pv_stash = []  # deferred single-qt PV thunks (ya-ring chain slack)

        def emit_pv_qt(hl, hf, kb, att, q0, lq, qt):
            c0 = max(0, qt * 512 - q0)
            c1 = min(lq, (qt + 1) * 512 - q0)
            o0 = q0 + c0 - qt * 512
            if (hl, qt) not in ya_tiles:
                ya_tiles[(hl, qt)] = ps_y.tile(
                    [P, 512], F32, tag="y", name=f"ya{hl}_{qt}"
                )
            ya = ya_tiles[(hl, qt)]
            nc.tensor.matmul(
                ya[0:VG, o0 : o0 + (c1 - c0)],
                vnat[:, kb, hl * VG : (hl + 1) * VG],
                att[:, c0:c1],
                start=(kb == 0),
                stop=(kb == 4 * qt + 3),
            )
            last_head_qt3 = hl == HL - 1 and qt == 3

            def emit_norm(cb, w):
                # pull y and the denominator straight out of PSUM so the ya
                # bank frees ~1us after the stop, independent of the
                # broadcast/multiply tail of the normalize chain
                rd = attn.tile([1, 512], F32, tag="rd", name="rd", bufs=3)
                nc.vector.reciprocal(rd[:, 0:w], ya[HD : HD + 1, cb : cb + w])
                yu = attn.tile([HD, 512], BF16, tag="yu", name="yu", bufs=3)
                nc.vector.tensor_copy(yu[:, 0:w], ya[0:HD, cb : cb + w])
                norm_q.append((hl, qt, yu, rd, cb, w))

            if last_head_qt3 and kb == 13:
                # columns 1536-1791 only attend to k-blocks <= 13, so their
                # slice of ya is final two stages before the qt stop: start
                # its normalize chain early to shorten the endgame tail
                emit_norm(0, 256)
            if kb == 4 * qt + 3:
                if last_head_qt3:
                    emit_norm(256, 256)
                else:
                    emit_norm(0, 512)

        def emit_pv(ent):
            hl, hf, kb, att, q0, lq = ent
            while pv_stash:
                pv_stash.pop(0)()
            qts = [qt for qt in (2 * hf, 2 * hf + 1) if kb <= 4 * qt + 3]
            if kb == 0 and len(qts) == 2:
                # defer the second qt one stage: its ya buffer is freed by a
                # normalize-multiply that is still in flight on DVE/Pool
                pv_stash.append(
                    lambda qt=qts[1]: emit_pv_qt(hl, hf, kb, att, q0, lq, qt)
                )
                qts = qts[:1]
            for qt in qts:
                emit_pv_qt(hl, hf, kb, att, q0, lq, qt)

        stages = [
            (hl, hf, kb)
            for hl in range(HL)
            for hf in (0, 1)
            for kb in range(8 if hf == 0 else 16)
        ]
        def emit_scores(sp, att_sl, kT, qT, kb, q0, lq, j0):
            # scores^T[k, q] into sp[:, j0:j0+lq], fp8 DoubleRow: the two
            # head-dim halves are the k-tiles, so each column costs 0.5
            # cycles -- half the bf16 streaming cost
            for j in range(0, lq, 512):
                f = min(512, lq - j)
                nc.tensor.matmul(
                    sp[:, j0 + j : j0 + j + f],
                    kT[:, :, ts(kb, P)],
                    qT[:, :, q0 + j : q0 + j + f],
                    start=True,
                    stop=True,
                    perf_mode=mybir.MatmulPerfMode.DoubleRow,
                )

        def emit_mask(att, kb, q0, j0):
            # diagonal block: zero out k > q entries
            if kb * P == q0:
                nc.vector.tensor_mul(
                    out=att[:, j0 : j0 + P],
                    in0=att[:, j0 : j0 + P],
                    in1=trimask,
                )

        # tail stages (lq <= 512) are emitted pairwise: both stages' scores
        # share one PSUM tile and a single exp, halving ACT dispatches there
        MERGE = {(0, 4): 5, (0, 6): 7, (1, 12): 13, (1, 14): 15}
        follower_entries = {}
        pending = deque()
        for hl, hf, kb in stages:
            p0 = (hl % 2) * HD
            fa = 32 * (hl % 3)
            qT = qkf[fa : fa + 32, 0, hl // 3]
            kT = qkf[fa : fa + 32, 1, hl // 3]
            q0 = max(kb * P, hf * 1024)
            lq = (hf + 1) * 1024 - q0
            if (hl, hf, kb) in follower_entries:
                ent = follower_entries.pop((hl, hf, kb))
            elif (hf, kb) in MERGE:
                kb2 = MERGE[(hf, kb)]
                q02 = kb2 * P
                lq2 = (hf + 1) * 1024 - q02
                sp = ps_s.tile([P, 1024], F32, tag="s")
                att = attn.tile([P, 1024], BF16, tag="att", bufs=5)
                emit_scores(sp, att, kT, qT, kb, q0, lq, 0)
                emit_scores(sp, att, kT, qT, kb2, q02, lq2, lq)
                nc.scalar.activation(
                    att[:, : lq + lq2], sp[:, : lq + lq2],
                    mybir.ActivationFunctionType.Exp, scale=scale,
                )
                emit_mask(att, kb, q0, 0)
                emit_mask(att, kb2, q02, lq)
                ent = (hl, hf, kb, att[:, 0:lq], q0, lq)
                follower_entries[(hl, hf, kb2)] = (
                    hl, hf, kb2, att[:, lq : lq + lq2], q02, lq2
                )
            else:
                sp = ps_s.tile([P, 1024], F32, tag="s")
                att = attn.tile([P, 1024], BF16, tag="att", bufs=5)
                emit_scores(sp, att, kT, qT, kb, q0, lq, 0)
                nc.scalar.activation(
                    att[:, :lq], sp[:, :lq],
                    mybir.ActivationFunctionType.Exp, scale=scale,
                )
                emit_mask(att, kb, q0, 0)
                ent = (hl, hf, kb, att[:, 0:lq], q0, lq)
            flush_norms()
            if len(pending) >= 3:
                emit_pv(pending.popleft())
            pending.append(ent)
            for fn in fillers.get((hl, hf, kb), ()):
                fn()

        # ---- drain: final PVs, then tt2 remainder while the qt3 norm chain
        # completes on DVE/Pool, then the qt3-dependent tt3 remainder
        while len(pending) > 1:
            emit_pv(pending.popleft())
            flush_norms()
        emit_pv(pending.popleft())   # (5,1,15): stops qt3, queues its norm
        while pv_stash:
            pv_stash.pop(0)()
        for ob in range(OUTB):
            emit_outproj(2, ob, [2], "b", "act" if ob % 2 else "dve")
        flush_norms()                # qt3 normalize
        for ob in range(OUTB):
            emit_outproj(3, ob, [2], "b", "act" if ob % 2 else "dve")


_NC_CACHE = None
LAST_RESULTS = None


def _get_nc():
    global _NC_CACHE
    if _NC_CACHE is None:
        _NC_CACHE = _build_bass()
    return _NC_CACHE


def kernel(x, W_attn, b_attn, W_o, b_o):
    global LAST_RESULTS
    x = np.asarray(x, np.float32)
    W_attn = np.asarray(W_attn, np.float32)
    b_attn = np.asarray(b_attn, np.float32)
    W_o = np.asarray(W_o, np.float32)
    b_o = np.asarray(b_o, np.float32)
    bf = ml_dtypes.bfloat16

    B = x.shape[0]
    in_maps = []
    for core in range(8):
        b, hg = divmod(core, 2)
        sl = slice(hg * J, (hg + 1) * J)
        wq = W_attn[0:C][sl]
        wk = W_attn[C : 2 * C][hg * J : (hg + 1) * J]
        wvl = W_attn[2 * C : 3 * C][hg * J : (hg + 1) * J]
        f8 = ml_dtypes.float8_e4m3
        xtb = np.ascontiguousarray(x[b].T)
        in_maps.append({
            "xt": xtb.astype(bf),
            "xt8": xtb.astype(f8),
            "wqk8": np.ascontiguousarray(
                np.concatenate([wq, wk], 0).T).astype(f8),
            "wv": np.ascontiguousarray(wvl.T).astype(bf),
            "wo": np.ascontiguousarray(W_o[:, sl].T).astype(bf),
            "bqk": np.ascontiguousarray(
                np.concatenate([b_attn[sl], b_attn[C + hg * J : C + (hg + 1) * J]])
            ),
            "bv": np.ascontiguousarray(b_attn[2 * C + hg * J : 2 * C + (hg + 1) * J]),
        })

    nc = _get_nc()
    LAST_RESULTS = bass_utils.run_bass_kernel_spmd(
        nc, in_maps, core_ids=list(range(8)),
        trace=bool(int(os.environ.get("KERNEL_TRACE", "0"))),
    )
    out = np.empty((B, T, C), np.float32)
    for b in range(B):
        acc = None
        for r in (LAST_RESULTS.results[2 * b], LAST_RESULTS.results[2 * b + 1]):
            for key in ("outa", "outb"):
                part = np.asarray(r[key])
                acc = part.astype(np.float32) if acc is None else acc + part
        out[b] = acc.T + b_o
    return out

